# revision 5
# baseline (speedup 1.0000x reference)
"""Trainium2 Bass kernel for nn_AudioDeviceModel (18-layer dilated causal CNN), v2.

Data-parallel over batch (64 = 8 cores x 8); per core one SBUF layout
[(b, chan) = 128 partitions, time = free dim].

Key restructurings vs the v1 baseline:
- The 1x1-conv residual recurrence  sig_{i+1} = h_i @ W_i + iob_i + 1*(s_i/2),
  s_i = 1^T sig_i, is folded into the next layer's conv (host-side weight
  products), removing the two full-width io/half matmuls per layer.  The
  scalar signal is kept as sigma_j = 8^-j s_j; its 3 taps ride in the ctrl
  matmul (rows 0..23), and its shifted tap copies are tiny SBUF->SBUF DMAs
  on the gpsimd ring with two-stage slack.
- The sigma update and the output mixer share one M=16 matmul per 2-tile
  chunk; the mixer accumulates across layers in an SBUF f32 accumulator via
  DVE adds (frees 4 PSUM banks for deeper conv buffering).
- The relu bias rides as lhsT row 32/96 against an all-ones rhs row.
- DMAs: ~850ns dispatch per 16 partitions on the issuing engine and ~45GB/s
  per ring, so blocks are fetched just-in-time, split across the sync and
  gpsimd rings; the scalar ring only carries the tiny stage-0 set before its
  eviction duty starts.  sigma_0 taps and the stage-0 conv come from a slim
  33-row x0 (3 tap matmuls) instead of a pre-im2col'd buffer.
bf16 matmuls, f32 PSUM.
"""

import numpy as np
import ml_dtypes

import concourse.bass as bass
import concourse.tile as tile
from concourse import bacc, mybir
from concourse.bass_utils import run_bass_kernel_spmd

# Problem constants (hardcoded; kernel.py must be self-contained).
DILATIONS = [1, 2, 4, 8, 16, 32, 64, 128, 256, 1, 2, 4, 8, 16, 32, 64, 128, 256]
UNIQ_DIL = [1, 2, 4, 8, 16, 32, 64, 128, 256]
KSIZE = 3
CH = 16
NUM_SIG = 1
NUM_CTRL = 3
FRAME = 2048
T = 4092
B = 64
NCORES = 8
BL = B // NCORES          # 8 batches per core
NL = len(DILATIONS)       # 18
TT = 512                  # time tile (= one PSUM bank of f32)
NTILES = (T + TT - 1) // TT
MIX_T0 = T - FRAME        # 2044

# Receptive-field trim: h_i only needed on [LO[i], T); LO[i] = 2*sum(d_j, j<=i)
# equals the exact validity frontier of the bias-fold, so no pad is ever read.
LO = []
_acc = 0
for _d in DILATIONS:
    _acc += 2 * _d
    LO.append(_acc)
assert LO[0] == 2 and LO[-1] == MIX_T0

BF16 = ml_dtypes.bfloat16

NWC = 384 + 17 * 512      # conv weight bank cols (stage0 3 taps + 17x[taps|ctrl])
NB = 17                   # c'_0..15 | mixer_b


def _wc_off(i):
    return 384 + (i - 1) * 512


def _build_weight_bank(conv_w0, conv_w, conv_b, io_w, io_b, mixer_w, mixer_b):
    """wbank [128, NWC] bf16 (conv lhsT; stage-0 block uses rows 0..32),
    wsm [128, NL*16] bf16 (supd+mix lhsT), bbank [8, NB] f32."""
    wbank = np.zeros((128, NWC), np.float32)
    wsm = np.zeros((128, NL * 40), np.float32)
    bbank = np.zeros((40, NB), np.float32)

    # stage 0 conv lhsT [33, 128] per tap: x0 rows 0..7 = signal(b),
    # rows 8..31 = ctrl (8 + b*3 + c), row 32 = ones (bias on tap 0)
    for k in range(KSIZE):
        o = k * 128
        for b in range(8):
            wbank[b, o + b * 16: o + (b + 1) * 16] = conv_w0[k, 0]
            for c in range(NUM_CTRL):
                wbank[8 + b * 3 + c, o + b * 16: o + (b + 1) * 16] = conv_w0[k, 1 + c]
            if k == 0:
                wbank[32, o + b * 16: o + (b + 1) * 16] = conv_b[0]

    for i in range(1, NL):
        o = _wc_off(i)
        w = conv_w[i - 1]                      # [3, 19, 16]
        Wio = io_w[i - 1]                      # [16, 16] h->sig
        bhat = conv_b[i].copy()
        for k in range(KSIZE):
            What = Wio @ w[k, :16]
            bhat += w[k, :16].T @ io_b[i - 1]
            for b in range(8):
                wbank[b * 16:(b + 1) * 16, o + k * 128 + b * 16:
                      o + k * 128 + (b + 1) * 16] = What
        # ctrl+s lhsT [97, 128]: rows 0..23 sigma taps (g*8+b, g=shift/d),
        # rows 24..95 ctrl taps, row 96 bias
        for b in range(8):
            for k in range(KSIZE):
                for c in range(NUM_CTRL):
                    wbank[24 + b * 9 + k * 3 + c,
                          o + 384 + b * 16: o + 384 + (b + 1) * 16] = w[k, CH + c]
                alpha = w[k, :16].sum(axis=0) / 2 * (8.0 ** (i - 1))
                g = KSIZE - 1 - k
                wbank[g * 8 + b, o + 384 + b * 16: o + 384 + (b + 1) * 16] = alpha
            wbank[96, o + 384 + b * 16: o + 384 + (b + 1) * 16] = bhat

    for i in range(NL):
        if i <= NL - 3:
            u = io_w[i].sum(axis=1) * (8.0 ** (-(i + 1)))
            for b in range(8):
                wsm[b * 16:(b + 1) * 16, i * 40 + b] = u
            bbank[:8, i] = io_b[i].sum() * (8.0 ** (-(i + 1)))
        for b in range(8):
            wsm[b * 16:(b + 1) * 16, i * 40 + 32 + b] = mixer_w[i * CH:(i + 1) * CH, 0]
    bbank[32:, 16] = mixer_b[0]
    return wbank.astype(BF16), wsm.astype(BF16), bbank


def _build_per_core_inputs(x_core):
    """x_core: [BL, T, 4] f32 -> (x0 [33, T] bf16, ctrlb [97, 9*T] bf16).

    x0: rows 0..7 = signal(b), rows 8..31 = ctrl (8+b*3+c), row 32 = 1.0.
    ctrlb rows 24..95: row 24+b*9+k*3+c of block di, col t = ctrl_c(b, t-(2-k)d).
    ctrlb row 96 = 1.0.  Rows 0..23 (sigma taps) are written on device.
    """
    x0 = np.zeros((33, T), np.float32)
    for b in range(BL):
        x0[b] = x_core[b, :, 0]
        for c in range(NUM_CTRL):
            x0[8 + b * 3 + c] = x_core[b, :, 1 + c]
    x0[32] = 1.0
    ctrl = x_core[:, :, NUM_SIG:]   # [BL, T, 3]
    ctrlb = np.zeros((97, len(UNIQ_DIL) * T), np.float32)
    ctrlb[96] = 1.0
    for di, d in enumerate(UNIQ_DIL):
        for k in range(KSIZE):
            sh = (KSIZE - 1 - k) * d
            for c in range(NUM_CTRL):
                for b in range(BL):
                    ctrlb[24 + b * 9 + k * 3 + c, di * T + sh: (di + 1) * T] = \
                        ctrl[b, : T - sh if sh else T, c]
    return x0.astype(BF16), ctrlb.astype(BF16)


def build_graph():
    nc = bacc.Bacc("TRN2", target_bir_lowering=False, debug=False)

    p_x0 = nc.declare_dram_parameter("x0b", [33, T], mybir.dt.bfloat16, isOutput=False)
    p_ctrl = nc.declare_dram_parameter(
        "ctrlb", [97, len(UNIQ_DIL) * T], mybir.dt.bfloat16, isOutput=False)
    p_w = nc.declare_dram_parameter("wbank", [128, NWC], mybir.dt.bfloat16, isOutput=False)
    p_wsm = nc.declare_dram_parameter("wsm", [128, NL * 40], mybir.dt.bfloat16, isOutput=False)
    p_b = nc.declare_dram_parameter("bbank", [40, NB], mybir.dt.float32, isOutput=False)
    p_out = nc.declare_dram_parameter("out", [8, FRAME], mybir.dt.float32, isOutput=True)

    di1 = UNIQ_DIL.index(DILATIONS[1])

    with tile.TileContext(nc) as tc:
        with (
            tc.tile_pool(name="persist", bufs=1) as persist,
            tc.tile_pool(name="hp", bufs=4, space="PSUM") as hp,
            tc.tile_pool(name="sp", bufs=2, space="PSUM") as sp,
        ):
            x0_sb = persist.tile([33, T], mybir.dt.bfloat16, tag="x0")
            ctrl_sb = persist.tile([97, len(UNIQ_DIL) * T], mybir.dt.bfloat16, tag="ctrl")
            w_sb = persist.tile([128, NWC], mybir.dt.bfloat16, tag="wbank")
            wsm_sb = persist.tile([128, NL * 40], mybir.dt.bfloat16, tag="wsm")
            b_sb = persist.tile([40, NB], mybir.dt.float32, tag="bbank")
            hA = persist.tile([128, T], mybir.dt.bfloat16, tag="hA")
            hB = persist.tile([128, T], mybir.dt.bfloat16, tag="hB")
            mix_acc = persist.tile([40, FRAME], mybir.dt.float32, tag="mixacc")
            out_sb = persist.tile([40, FRAME], mybir.dt.float32, tag="outsb")
            warm = persist.tile([128, TT], mybir.dt.bfloat16, tag="warm")

            nc.vector.memset(warm[:], 0.0)
            nc.vector.memset(mix_acc[:], 0.0)

            seen = set()

            def dma_ctrl(t):
                # sigma rows 0..23 are device-written; only ship ctrl+ones,
                # and only the columns its consumer stages can read
                # (cols >= min LO over users).  Early blocks (consumed while
                # the DMA rings are still crunched) go in two column chunks
                # so the first tiles unblock on partial arrival.
                di = UNIQ_DIL.index(DILATIONS[t])
                if di in seen:
                    return
                seen.add(di)
                lo_col = min(LO[i] for i in range(1, NL)
                             if DILATIONS[i] == DILATIONS[t])
                cuts = [lo_col, (lo_col + T) // 2, T] if t <= 4 else [lo_col, T]
                for c0, c1 in zip(cuts, cuts[1:]):
                    nc.sync.dma_start(
                        out=ctrl_sb[24:60, di * T + c0: di * T + c1],
                        in_=p_ctrl[24:60, di * T + c0: di * T + c1])
                    nc.gpsimd.dma_start(
                        out=ctrl_sb[60:, di * T + c0: di * T + c1],
                        in_=p_ctrl[60:, di * T + c0: di * T + c1])

            def dma_w(t, span=1):
                # one descriptor per partition row, so fetching `span`
                # adjacent stage blocks in one DMA is nearly free
                o = _wc_off(t)
                hi = o + span * 512
                nc.sync.dma_start(out=w_sb[:64, o:hi], in_=p_w[:64, o:hi])
                nc.gpsimd.dma_start(out=w_sb[64:, o:hi], in_=p_w[64:, o:hi])

            # startup-critical set, in need order, on the sync+gpsimd rings
            # only (the scalar ring's DMA path is slow; it just does evicts)
            nc.sync.dma_start(out=w_sb[:17, 0:384], in_=p_w[:17, 0:384])
            nc.gpsimd.dma_start(out=w_sb[17:33, 0:384], in_=p_w[17:33, 0:384])
            nc.sync.dma_start(out=x0_sb[:16, :], in_=p_x0[:16, :])
            nc.gpsimd.dma_start(out=x0_sb[16:, :], in_=p_x0[16:, :])
            # sigma_0 taps for stage 1 (d=2): shifted copies of the signal rows
            for g in range(KSIZE):
                nc.gpsimd.dma_start(
                    out=ctrl_sb[g * 8:(g + 1) * 8, di1 * T + 2 * g: di1 * T + T],
                    in_=x0_sb[0:8, 0: T - 2 * g])
            nc.sync.dma_start(out=b_sb[:8, :], in_=p_b[:8, :])
            nc.gpsimd.dma_start(out=b_sb[32:, :], in_=p_b[32:, :])
            nc.sync.dma_start(out=wsm_sb[:64, :], in_=p_wsm[:64, :])
            nc.gpsimd.dma_start(out=wsm_sb[64:, :], in_=p_wsm[64:, :])
            dma_ctrl(1)
            dma_w(1)
            dma_w(2)
            dma_ctrl(2)
            dma_ctrl(3)
            dma_w(3)
            dma_ctrl(4)

            # PE warm-up: p-state ramp + hold through the DMA-bound window.
            for _ in range(8):
                wps = hp.tile([128, TT], mybir.dt.float32, tag="hps")
                nc.tensor.matmul(wps[:, :128], warm[:, :128], warm[:, :128],
                                 start=True, stop=True)
                nc.tensor.matmul(wps[:, :128], warm[:, :128], warm[:, :128],
                                 start=True, stop=True)
            for _ in range(14):
                wps = hp.tile([128, TT], mybir.dt.float32, tag="hps")
                nc.tensor.matmul(wps[:, :TT], warm[:, :128], warm[:, :TT],
                                 start=True, stop=True)

            rings = [hA, hB]
            for i in range(NL):
                if 1 <= i and i + 3 < NL:
                    dma_ctrl(i + 3)
                    if (i + 3) % 2 == 0:
                        dma_w(i + 3, span=2 if i + 4 < NL else 1)
                d = DILATIONS[i]
                hcur = rings[i % 2]
                hprev = rings[(i - 1) % 2]
                o = _wc_off(i) if i >= 1 else 0
                di = UNIQ_DIL.index(d)

                lo = LO[i]
                # supd+mix tiles, in 2-tile chunks sharing one [40, 1024] psum
                # (rows 0..7 sigma partial, rows 32..39 mixer partial).
                # Stage i's chunks are emitted trailing stage i+1's conv
                # stream, giving their wsm/sigma/psum dependencies a full
                # stage of slack.
                rlo = LO[i + 1] if i <= NL - 3 else MIX_T0
                sup_tiles = list(range(rlo // TT, NTILES))
                chunks = [(i, tuple(sup_tiles[x:x + 2]), rlo, hcur)
                          for x in range(0, len(sup_tiles), 2)]
                emitted = 0

                def emit_chunk(spec):
                    ci, c, crlo, chsrc = spec
                    a_lo = max(c[0] * TT, crlo)
                    b_hi = min((c[-1] + 1) * TT, T)
                    s_ps = sp.tile([40, 2 * TT], mybir.dt.float32, tag="sps")
                    for j in c:
                        a = max(j * TT, crlo)
                        b = min((j + 1) * TT, T)
                        nc.tensor.matmul(
                            s_ps[:, a - a_lo: b - a_lo],
                            wsm_sb[:, ci * 40:(ci + 1) * 40],
                            chsrc[:, a:b],
                            start=True, stop=True,
                        )
                    if ci <= NL - 3:
                        src_bi = UNIQ_DIL.index(DILATIONS[ci + 1])
                        dst_bi = UNIQ_DIL.index(DILATIONS[ci + 2])
                        nc.vector.scalar_tensor_tensor(
                            out=ctrl_sb[0:8, dst_bi * T + a_lo: dst_bi * T + b_hi],
                            in0=s_ps[:8, :b_hi - a_lo],
                            scalar=b_sb[:8, ci:ci + 1],
                            in1=ctrl_sb[0:8, src_bi * T + a_lo: src_bi * T + b_hi],
                            op0=mybir.AluOpType.add,
                            op1=mybir.AluOpType.add,
                        )
                    m_lo = max(a_lo, MIX_T0)
                    if b_hi > m_lo:
                        nc.vector.tensor_add(
                            out=mix_acc[32:40, m_lo - MIX_T0: b_hi - MIX_T0],
                            in0=mix_acc[32:40, m_lo - MIX_T0: b_hi - MIX_T0],
                            in1=s_ps[32:40, m_lo - a_lo: b_hi - a_lo],
                        )

                for j in range(lo // TT, NTILES):
                    a = max(j * TT, lo)
                    b = min((j + 1) * TT, T)
                    w = b - a
                    h_ps = hp.tile([128, TT], mybir.dt.float32, tag="hps")
                    if i == 0:
                        for k in range(KSIZE):
                            shift = KSIZE - 1 - k
                            nc.tensor.matmul(
                                h_ps[:, :w],
                                w_sb[:33, k * 128:(k + 1) * 128],
                                x0_sb[:, a - shift: b - shift],
                                start=(k == 0), stop=(k == KSIZE - 1),
                            )
                    else:
                        for k in range(KSIZE):
                            shift = (KSIZE - 1 - k) * d
                            nc.tensor.matmul(
                                h_ps[:, :w],
                                w_sb[:, o + k * 128: o + (k + 1) * 128],
                                hprev[:, a - shift: b - shift],
                                start=(k == 0), stop=False,
                            )
                        nc.tensor.matmul(
                            h_ps[:, :w],
                            w_sb[:97, o + 384: o + 512],
                            ctrl_sb[:, di * T + a: di * T + b],
                            start=False, stop=True,
                        )
                    # h = relu(psum) straight into the ring (bias already in)
                    nc.scalar.activation(
                        out=hcur[:, a:b],
                        in_=h_ps[:, :w],
                        func=mybir.ActivationFunctionType.Relu,
                        scale=1.0,
                    )
                    # trail the conv stream by one full tile
                    while emitted < len(chunks) and chunks[emitted][1][-1] <= j - 1:
                        emit_chunk(chunks[emitted])
                        emitted += 1
                while emitted < len(chunks):
                    emit_chunk(chunks[emitted])
                    emitted += 1
                if i <= NL - 3:
                    # shifted tap copies for stage i+2, two-stage slack
                    d2 = DILATIONS[i + 2]
                    dst_bi = UNIQ_DIL.index(d2)
                    lo1 = LO[i + 1]
                    nc.gpsimd.dma_start(
                        out=ctrl_sb[8:16, dst_bi * T + lo1 + d2: dst_bi * T + T],
                        in_=ctrl_sb[0:8, dst_bi * T + lo1: dst_bi * T + T - d2])
                    nc.gpsimd.dma_start(
                        out=ctrl_sb[16:24, dst_bi * T + lo1 + 2 * d2: dst_bi * T + T],
                        in_=ctrl_sb[0:8, dst_bi * T + lo1: dst_bi * T + T - 2 * d2])

            # final: out = mix_acc + mixer_b, four pipelined chunks
            # alternating ScalarE/VectorE, each DMA'd out once evicted
            for ci in range(4):
                c0, c1 = ci * 512, (ci + 1) * 512
                if ci % 2 == 0:
                    nc.scalar.activation(
                        out=out_sb[32:40, c0:c1],
                        in_=mix_acc[32:40, c0:c1],
                        func=mybir.ActivationFunctionType.Identity,
                        bias=b_sb[32:40, 16:17],
                        scale=1.0,
                    )
                else:
                    nc.vector.tensor_scalar_add(
                        out=out_sb[32:40, c0:c1],
                        in0=mix_acc[32:40, c0:c1],
                        scalar1=b_sb[32:40, 16:17],
                    )
                (nc.sync if ci % 2 == 0 else nc.gpsimd).dma_start(
                    out=p_out[:, c0:c1], in_=out_sb[32:40, c0:c1])

    nc.finalize()
    return nc


_CACHE = {}


def kernel(**inputs) -> np.ndarray:
    inp = inputs["input"].astype(np.float32)          # [64, 4092, 4]
    wbank, wsm, bbank = _build_weight_bank(
        inputs["conv_w0"].astype(np.float32),
        inputs["conv_w"].astype(np.float32),
        inputs["conv_b"].astype(np.float32),
        inputs["io_w"].astype(np.float32),
        inputs["io_b"].astype(np.float32),
        inputs["mixer_w"].astype(np.float32),
        inputs["mixer_b"].astype(np.float32),
    )

    if "nc" not in _CACHE:
        _CACHE["nc"] = build_graph()
    nc = _CACHE["nc"]

    in_maps = []
    for c in range(NCORES):
        x0b, ctrlb = _build_per_core_inputs(inp[c * BL:(c + 1) * BL])
        in_maps.append({"x0b": x0b, "ctrlb": ctrlb, "wbank": wbank,
                        "wsm": wsm, "bbank": bbank})

    res = run_bass_kernel_spmd(nc, in_maps, core_ids=list(range(NCORES)))
    outs = [res.results[c]["out"] for c in range(NCORES)]       # each [8, 2048]
    full = np.concatenate(outs, axis=0)                         # [64, 2048]
    return full[:, :, None].astype(np.float32)                  # [64, 2048, 1]


if __name__ == "__main__":
    data = np.load("/root/problem/ref_inputs.npz")
    out = kernel(**{k: data[k] for k in data.files})
    ref = np.load("/root/problem/ref_out.npy")
    err = np.linalg.norm(out - ref) / np.linalg.norm(ref)
    print("Relative error:", err)
